# revision 7
# baseline (speedup 1.0000x reference)
"""v3: symmetry + fp8e4m3 DoubleRow GEMM for the denominator.

Same structure as kernel2 (rotation, 5 column groups, rowsum/colsum/pos
partials, host combine), but the similarity GEMM runs in fp8 with
perf_mode=DoubleRow: z is quantized to e4m3, bounced through DRAM as uint16
byte-pairs, xbar-transposed (u16), and contracted 256 d per matmul via the
3D [Ki, 2, N] pair AP. Positives stay on the bf16 path (precision), so only
the exp-sum denominators see fp8 noise, which averages out over 8191 terms.
"""

import numpy as np

try:
    import concourse.bass as bass
except ImportError:
    import sys

    for _p in ("/opt/trn_rl_repo", "/root/.axon_site/_ro/trn_rl_repo"):
        if _p not in sys.path:
            sys.path.append(_p)
    import concourse.bass as bass

import concourse.mybir as mybir
import concourse.tile as tile
from concourse import bacc
from concourse.bass_utils import run_bass_kernel_spmd

F32 = mybir.dt.float32
BF16 = mybir.dt.bfloat16
FP8 = mybir.dt.float8e4
U16 = mybir.dt.uint16
AF = mybir.ActivationFunctionType
ALU = mybir.AluOpType

B = 4096
D = 1024
R = 2 * B
N_CORES = 8
LOCAL = R // N_CORES
INV_TEMP = 2.0
E2 = float(np.exp(INV_TEMP))

NGC = 5
KK = D // 256      # 4 contraction chunks of 256 (DoubleRow pairs)
CS_G = (1, 2, 3)


def build_nc():
    nc = bacc.Bacc("TRN2", target_bir_lowering=False, debug=False)

    embr = nc.dram_tensor("embr", [NGC * 1024, D], F32, kind="ExternalInput")
    out_rowsum = nc.dram_tensor("rowsum", [128, 8], F32, kind="ExternalOutput")
    out_pos = nc.dram_tensor("pos", [128, 8], F32, kind="ExternalOutput")
    out_colsum = nc.dram_tensor("colsum", [1, 3072], F32, kind="ExternalOutput")

    with tile.TileContext(nc) as tc:
        with (
            tc.tile_pool(name="xin", bufs=4) as xin_pool,
            tc.tile_pool(name="zrow", bufs=4) as zrow_pool,
            tc.tile_pool(name="zkeep", bufs=8) as zkeep_pool,
            tc.tile_pool(name="stats", bufs=16) as stats_pool,
            tc.tile_pool(name="sqj", bufs=3) as sqj_pool,
            tc.tile_pool(name="zt", bufs=1) as zt_pool,
            tc.tile_pool(name="acc", bufs=1) as acc_pool,
            tc.tile_pool(name="ej", bufs=6) as ej_pool,
            tc.tile_pool(name="dram", bufs=1, space="DRAM") as dram_pool,
            tc.tile_pool(name="psum", bufs=6, space="PSUM") as psum_pool,
            tc.tile_pool(name="pscs", bufs=2, space="PSUM") as pscs_pool,
        ):
            # transposed fp8 z as u16 byte-pairs: ztu[kk][pair_p, row]
            # covers d = 256*kk + 2*p + i
            ztus = [
                zt_pool.tile([128, NGC * 1024], U16, tag=f"ztu{kk}", name=f"ztu{kk}")
                for kk in range(KK)
            ]
            zbufs = [
                dram_pool.tile([1024, D // 2], U16, tag=f"zb{g}", name=f"zb{g}")
                for g in range(NGC)
            ]

            pos = acc_pool.tile([128, 8], F32, name="pos")
            rs = acc_pool.tile([128, 8, 2 * NGC], F32, name="rs")
            colsum_acc = acc_pool.tile([1, 3072], F32, name="colsum_acc")
            ones = acc_pool.tile([128, 1], BF16, name="ones")
            nc.vector.memset(ones[:], 1.0)

            keep = [None] * 8

            # ---- Phase 1: normalize, quantize to fp8, bounce, u16-transpose
            for g in range(NGC):
                for tl in range(8):
                    t = g * 8 + tl
                    x = xin_pool.tile([128, D], F32, tag="x", name=f"x{t}")
                    nc.sync.dma_start(out=x[:], in_=embr[t * 128 : (t + 1) * 128, :])

                    sqj = sqj_pool.tile([128, D], F32, tag="sqj", name=f"sqj{t}")
                    ssq = stats_pool.tile([128, 1], F32, tag="ssq", name=f"ssq{t}")
                    nc.scalar.activation(
                        out=sqj[:], in_=x[:], func=AF.Square, accum_out=ssq[:]
                    )
                    nrm = stats_pool.tile([128, 1], F32, tag="nrm", name=f"nrm{t}")
                    nc.scalar.sqrt(nrm[:], ssq[:])
                    inv = stats_pool.tile([128, 1], F32, tag="inv", name=f"inv{t}")
                    nc.vector.reciprocal(out=inv[:], in_=nrm[:])

                    # fp8 path (denominator GEMM)
                    z8 = zrow_pool.tile([128, D], FP8, tag="z8", name=f"z8_{t}")
                    nc.vector.tensor_scalar_mul(z8[:], x[:], inv[:])
                    nc.sync.dma_start(
                        out=zbufs[g][tl * 128 : (tl + 1) * 128, :],
                        in_=z8.bitcast(U16)[:],
                    )

                    # bf16 path (positives only)
                    if t < 8:
                        zb = zkeep_pool.tile([128, D], BF16, tag="zk", name=f"zk{t}")
                        keep[t] = zb
                        nc.vector.tensor_scalar_mul(zb[:], x[:], inv[:])
                    elif 32 <= t < 40:
                        zb = zrow_pool.tile([128, D], BF16, tag="zr", name=f"zr{t}")
                        nc.vector.tensor_scalar_mul(zb[:], x[:], inv[:])
                        tt = t - 32
                        pj = sqj_pool.tile([128, D], F32, tag="pj", name=f"pj{t}")
                        nc.vector.tensor_mul(pj[:], keep[tt][:], zb[:])
                        nc.vector.tensor_reduce(
                            out=pos[:, tt : tt + 1],
                            in_=pj[:],
                            axis=mybir.AxisListType.X,
                            op=ALU.add,
                        )

                for kk in range(KK):
                    nc.sync.dma_start(
                        out=ztus[kk][:, g * 1024 : (g + 1) * 1024],
                        in_=zbufs[g][:, kk * 128 : (kk + 1) * 128],
                        transpose=True,
                    )

            # deinterleaved fp8 pair tiles [128, 2, NGC*1024]: byte-interleaved
            # pair strides fail the LDWEIGHTS ISA check, so DVE-copy into the
            # tile_matmul-style layout (pair stride = NGC*1024 bytes).
            ztds = [
                zt_pool.tile([128, 2, NGC * 1024], FP8, tag=f"ztd{kk}", name=f"ztd{kk}")
                for kk in range(KK)
            ]
            for kk in range(KK):
                for g in range(NGC):
                    src = ztus[kk].bitcast(FP8)[
                        :, g * 2048 : (g + 1) * 2048
                    ].rearrange("p (n two) -> p two n", two=2)
                    nc.vector.tensor_copy(
                        ztds[kk][:, :, g * 1024 : (g + 1) * 1024], src
                    )
            z3s = [ztds[kk] for kk in range(KK)]

            # ---- Phase 2: fp8 DoubleRow GEMM + exp/rowsum (+ colsum for g 1..3)
            for g in range(NGC):
                want_cs = g in CS_G
                css = None
                if want_cs:
                    css = [
                        pscs_pool.tile([1, 512], F32, tag="cs", name=f"cs{g}_{cb}")
                        for cb in range(2)
                    ]
                for m in range(8):
                    pss = [
                        psum_pool.tile([128, 512], F32, tag="ps", name=f"ps{g}_{m}_{cb}")
                        for cb in range(2)
                    ]
                    for kk in range(KK):
                        lhsT = z3s[kk][:, :, m * 128 : (m + 1) * 128]
                        for cb in range(2):
                            nc.tensor.matmul(
                                pss[cb][:],
                                lhsT,
                                z3s[kk][
                                    :, :, g * 1024 + cb * 512 : g * 1024 + (cb + 1) * 512
                                ],
                                start=(kk == 0),
                                stop=(kk == KK - 1),
                                perf_mode=mybir.MatmulPerfMode.DoubleRow,
                            )
                    for cb in range(2):
                        ej = ej_pool.tile(
                            [128, 512], BF16, tag="ej", name=f"ej{g}_{m}_{cb}"
                        )
                        j = g * 2 + cb
                        nc.scalar.activation(
                            out=ej[:],
                            in_=pss[cb][:],
                            func=AF.Exp,
                            bias=0.0,
                            scale=INV_TEMP,
                            accum_out=rs[:, m, j : j + 1],
                        )
                        if want_cs:
                            nc.tensor.matmul(
                                css[cb][:],
                                ones[:],
                                ej[:],
                                start=(m == 0),
                                stop=(m == 7),
                            )
                if want_cs:
                    for cb in range(2):
                        off = (g - 1) * 1024 + cb * 512
                        nc.vector.tensor_copy(colsum_acc[:, off : off + 512], css[cb][:])

            # ---- Phase 3: write partial outputs
            rowsum = acc_pool.tile([128, 8], F32, name="rowsum")
            nc.vector.tensor_reduce(
                out=rowsum[:], in_=rs[:], axis=mybir.AxisListType.X, op=ALU.add
            )
            nc.sync.dma_start(out=out_rowsum[:, :], in_=rowsum[:])
            nc.sync.dma_start(out=out_pos[:, :], in_=pos[:])
            nc.sync.dma_start(out=out_colsum[:, :], in_=colsum_acc[:])

    nc.compile()
    return nc


_NC = None


def _get_nc():
    global _NC
    if _NC is None:
        _NC = build_nc()
    return _NC


def make_in_maps(emb_i, emb_j):
    reps = np.concatenate(
        [np.asarray(emb_i, np.float32), np.asarray(emb_j, np.float32)], axis=0
    )
    rolled = [np.roll(reps, -c * LOCAL, axis=0)[: NGC * 1024] for c in range(N_CORES)]
    return [{"embr": np.ascontiguousarray(r)} for r in rolled]


def run_spmd(in_maps, trace=False, **kwargs):
    return run_bass_kernel_spmd(
        _get_nc(), in_maps, core_ids=list(range(N_CORES)), trace=trace, **kwargs
    )


def combine(results):
    rowsum = np.stack(
        [r["rowsum"].astype(np.float64).T.reshape(LOCAL) for r in results]
    )
    pos = np.stack([r["pos"].astype(np.float64).T.reshape(LOCAL) for r in results])
    colsum = np.stack(
        [r["colsum"].astype(np.float64).reshape(3, 1024) for r in results]
    )
    denom = rowsum.copy()
    for b in range(N_CORES):
        for gp in CS_G:
            denom[b] += colsum[(b - gp) % N_CORES][gp - 1]
    denom -= E2
    loss_rows = np.log(denom) - INV_TEMP * pos
    return float(loss_rows.sum() / R)


def kernel(emb_i, emb_j):
    res = run_spmd(make_in_maps(emb_i, emb_j))
    return np.array(combine(res.results), dtype=np.float32)


# revision 8
# speedup vs baseline: 1.0749x; 1.0749x over previous
"""v3: symmetry + fp8e4m3 DoubleRow GEMM for the denominator.

Same structure as kernel2 (rotation, 5 column groups, rowsum/colsum/pos
partials, host combine), but the similarity GEMM runs in fp8 with
perf_mode=DoubleRow: z is quantized to e4m3, bounced through DRAM as uint16
byte-pairs, xbar-transposed (u16), and contracted 256 d per matmul via the
3D [Ki, 2, N] pair AP. Positives stay on the bf16 path (precision), so only
the exp-sum denominators see fp8 noise, which averages out over 8191 terms.
"""

import numpy as np

try:
    import concourse.bass as bass
except ImportError:
    import sys

    for _p in ("/opt/trn_rl_repo", "/root/.axon_site/_ro/trn_rl_repo"):
        if _p not in sys.path:
            sys.path.append(_p)
    import concourse.bass as bass

import concourse.mybir as mybir
import concourse.tile as tile
from concourse import bacc
from concourse.bass_utils import run_bass_kernel_spmd

F32 = mybir.dt.float32
BF16 = mybir.dt.bfloat16
FP8 = mybir.dt.float8e4
U16 = mybir.dt.uint16
AF = mybir.ActivationFunctionType
ALU = mybir.AluOpType

B = 4096
D = 1024
R = 2 * B
N_CORES = 8
LOCAL = R // N_CORES
INV_TEMP = 2.0
E2 = float(np.exp(INV_TEMP))

NGC = 5
KK = D // 256      # 4 contraction chunks of 256 (DoubleRow pairs)
CS_G = (1, 2, 3)


def build_nc():
    nc = bacc.Bacc("TRN2", target_bir_lowering=False, debug=False)

    embr = nc.dram_tensor("embr", [NGC * 1024, D], F32, kind="ExternalInput")
    out_rowsum = nc.dram_tensor("rowsum", [128, 8], F32, kind="ExternalOutput")
    out_pos = nc.dram_tensor("pos", [128, 8], F32, kind="ExternalOutput")
    out_colsum = nc.dram_tensor("colsum", [1, 3072], F32, kind="ExternalOutput")

    with tile.TileContext(nc) as tc:
        with (
            tc.tile_pool(name="xin", bufs=6) as xin_pool,
            tc.tile_pool(name="zrow", bufs=4) as zrow_pool,
            tc.tile_pool(name="zkeep", bufs=8) as zkeep_pool,
            tc.tile_pool(name="stats", bufs=16) as stats_pool,
            tc.tile_pool(name="sqj", bufs=3) as sqj_pool,
            tc.tile_pool(name="zt", bufs=1) as zt_pool,
            tc.tile_pool(name="acc", bufs=1) as acc_pool,
            tc.tile_pool(name="ej", bufs=6) as ej_pool,
            tc.tile_pool(name="dram", bufs=1, space="DRAM") as dram_pool,
            tc.tile_pool(name="psum", bufs=6, space="PSUM") as psum_pool,
            tc.tile_pool(name="pscs", bufs=2, space="PSUM") as pscs_pool,
        ):
            # transposed fp8 z as u16 byte-pairs: ztu[kk][pair_p, row]
            # covers d = 256*kk + 2*p + i
            ztus = [
                zt_pool.tile([128, NGC * 1024], U16, tag=f"ztu{kk}", name=f"ztu{kk}")
                for kk in range(KK)
            ]
            zbufs = [
                dram_pool.tile([1024, D // 2], U16, tag=f"zb{g}", name=f"zb{g}")
                for g in range(NGC)
            ]

            pos = acc_pool.tile([128, 8], F32, name="pos")
            rs = acc_pool.tile([128, 8, 2 * NGC], F32, name="rs")
            colsum_acc = acc_pool.tile([1, 3072], F32, name="colsum_acc")
            ones = acc_pool.tile([128, 1], BF16, name="ones")
            nc.vector.memset(ones[:], 1.0)

            keep = [None] * 8

            # ---- Phase 1: normalize, quantize to fp8, bounce, u16-transpose
            for g in range(NGC):
                for tl in range(8):
                    t = g * 8 + tl
                    x = xin_pool.tile([128, D], F32, tag="x", name=f"x{t}")
                    nc.sync.dma_start(out=x[:], in_=embr[t * 128 : (t + 1) * 128, :])

                    sqj = sqj_pool.tile([128, D], F32, tag="sqj", name=f"sqj{t}")
                    ssq = stats_pool.tile([128, 1], F32, tag="ssq", name=f"ssq{t}")
                    nc.scalar.activation(
                        out=sqj[:], in_=x[:], func=AF.Square, accum_out=ssq[:]
                    )
                    nrm = stats_pool.tile([128, 1], F32, tag="nrm", name=f"nrm{t}")
                    nc.scalar.sqrt(nrm[:], ssq[:])
                    inv = stats_pool.tile([128, 1], F32, tag="inv", name=f"inv{t}")
                    nc.vector.reciprocal(out=inv[:], in_=nrm[:])

                    # fp8 path (denominator GEMM)
                    z8 = zrow_pool.tile([128, D], FP8, tag="z8", name=f"z8_{t}")
                    nc.vector.tensor_scalar_mul(z8[:], x[:], inv[:])
                    nc.sync.dma_start(
                        out=zbufs[g][tl * 128 : (tl + 1) * 128, :],
                        in_=z8.bitcast(U16)[:],
                    )

                    # bf16 path (positives only)
                    if t < 8:
                        zb = zkeep_pool.tile([128, D], BF16, tag="zk", name=f"zk{t}")
                        keep[t] = zb
                        nc.vector.tensor_scalar_mul(zb[:], x[:], inv[:])
                    elif 32 <= t < 40:
                        zb = zrow_pool.tile([128, D], BF16, tag="zr", name=f"zr{t}")
                        nc.vector.tensor_scalar_mul(zb[:], x[:], inv[:])
                        tt = t - 32
                        pj = sqj_pool.tile([128, D], F32, tag="pj", name=f"pj{t}")
                        nc.vector.tensor_mul(pj[:], keep[tt][:], zb[:])
                        nc.vector.tensor_reduce(
                            out=pos[:, tt : tt + 1],
                            in_=pj[:],
                            axis=mybir.AxisListType.X,
                            op=ALU.add,
                        )

                for kk in range(KK):
                    nc.sync.dma_start(
                        out=ztus[kk][:, g * 1024 : (g + 1) * 1024],
                        in_=zbufs[g][:, kk * 128 : (kk + 1) * 128],
                        transpose=True,
                    )

            # deinterleaved fp8 pair tiles [128, 2, NGC*1024]: byte-interleaved
            # pair strides fail the LDWEIGHTS ISA check, so DVE-copy into the
            # tile_matmul-style layout (pair stride = NGC*1024 bytes).
            ztds = [
                zt_pool.tile([128, 2, NGC * 1024], FP8, tag=f"ztd{kk}", name=f"ztd{kk}")
                for kk in range(KK)
            ]
            for kk in range(KK):
                for g in range(NGC):
                    src = ztus[kk].bitcast(FP8)[
                        :, g * 2048 : (g + 1) * 2048
                    ].rearrange("p (n two) -> p two n", two=2)
                    nc.vector.tensor_copy(
                        ztds[kk][:, :, g * 1024 : (g + 1) * 1024], src
                    )
            z3s = [ztds[kk] for kk in range(KK)]

            # ---- Phase 2: fp8 DoubleRow GEMM + exp/rowsum (+ colsum for g 1..3)
            for g in range(NGC):
                want_cs = g in CS_G
                css = None
                if want_cs:
                    css = [
                        pscs_pool.tile([1, 512], F32, tag="cs", name=f"cs{g}_{cb}")
                        for cb in range(2)
                    ]
                for m in range(8):
                    pss = [
                        psum_pool.tile([128, 512], F32, tag="ps", name=f"ps{g}_{m}_{cb}")
                        for cb in range(2)
                    ]
                    for kk in range(KK):
                        lhsT = z3s[kk][:, :, m * 128 : (m + 1) * 128]
                        for cb in range(2):
                            nc.tensor.matmul(
                                pss[cb][:],
                                lhsT,
                                z3s[kk][
                                    :, :, g * 1024 + cb * 512 : g * 1024 + (cb + 1) * 512
                                ],
                                start=(kk == 0),
                                stop=(kk == KK - 1),
                                perf_mode=mybir.MatmulPerfMode.DoubleRow,
                            )
                    for cb in range(2):
                        ej = ej_pool.tile(
                            [128, 512], BF16, tag="ej", name=f"ej{g}_{m}_{cb}"
                        )
                        j = g * 2 + cb
                        nc.scalar.activation(
                            out=ej[:],
                            in_=pss[cb][:],
                            func=AF.Exp,
                            bias=0.0,
                            scale=INV_TEMP,
                            accum_out=rs[:, m, j : j + 1],
                        )
                        if want_cs:
                            nc.tensor.matmul(
                                css[cb][:],
                                ones[:],
                                ej[:],
                                start=(m == 0),
                                stop=(m == 7),
                            )
                if want_cs:
                    for cb in range(2):
                        off = (g - 1) * 1024 + cb * 512
                        nc.vector.tensor_copy(colsum_acc[:, off : off + 512], css[cb][:])

            # ---- Phase 3: write partial outputs
            rowsum = acc_pool.tile([128, 8], F32, name="rowsum")
            nc.vector.tensor_reduce(
                out=rowsum[:], in_=rs[:], axis=mybir.AxisListType.X, op=ALU.add
            )
            nc.sync.dma_start(out=out_rowsum[:, :], in_=rowsum[:])
            nc.sync.dma_start(out=out_pos[:, :], in_=pos[:])
            nc.sync.dma_start(out=out_colsum[:, :], in_=colsum_acc[:])

    nc.compile()
    return nc


_NC = None


def _get_nc():
    global _NC
    if _NC is None:
        _NC = build_nc()
    return _NC


def make_in_maps(emb_i, emb_j):
    reps = np.concatenate(
        [np.asarray(emb_i, np.float32), np.asarray(emb_j, np.float32)], axis=0
    )
    rolled = [np.roll(reps, -c * LOCAL, axis=0)[: NGC * 1024] for c in range(N_CORES)]
    return [{"embr": np.ascontiguousarray(r)} for r in rolled]


def run_spmd(in_maps, trace=False, **kwargs):
    return run_bass_kernel_spmd(
        _get_nc(), in_maps, core_ids=list(range(N_CORES)), trace=trace, **kwargs
    )


def combine(results):
    rowsum = np.stack(
        [r["rowsum"].astype(np.float64).T.reshape(LOCAL) for r in results]
    )
    pos = np.stack([r["pos"].astype(np.float64).T.reshape(LOCAL) for r in results])
    colsum = np.stack(
        [r["colsum"].astype(np.float64).reshape(3, 1024) for r in results]
    )
    denom = rowsum.copy()
    for b in range(N_CORES):
        for gp in CS_G:
            denom[b] += colsum[(b - gp) % N_CORES][gp - 1]
    denom -= E2
    loss_rows = np.log(denom) - INV_TEMP * pos
    return float(loss_rows.sum() / R)


def kernel(emb_i, emb_j):
    res = run_spmd(make_in_maps(emb_i, emb_j))
    return np.array(combine(res.results), dtype=np.float32)
